# revision 15
# baseline (speedup 1.0000x reference)
"""SecGELU table-lookup kernel for Trainium2 (8 NeuronCores, data-parallel).

Reference semantics (per element):
    a = |x|; c = min(int(a * 1024), 4095); out = relu(x) - table[c]

Device algorithm
----------------
A 4096-way per-element gather has no line-rate engine on TRN2 (GpSimd
gathers share indices across 16-partition groups and run ~100x too slow),
but the table produced by the model is exactly T[j] = relu(j/1024) -
gelu_erf(j/1024).  For q >= 0:  gelu(-q) = -q * Phi(-q) = -(q - gelu(q)) =
-T[q*1024], so

    out = relu(x) + Gelu(-q),   q = min(floor(|x|*1024), 4095) / 1024

maps the whole lookup onto one ACT-engine Gelu pass.  The host verifies
the runtime table against the erf-GELU generator before using this
identity; on mismatch it falls back to an exact host-side gather (never
taken for the real model table).

Quantization is reproduced with fp32 tricks, all stock ops:
  t   = |x| * 1024                      (ACT: Abs, scale=1024; exact)
  y   = min(t, 4095.5) + (2^23 - 0.5)   (DVE tensor_scalar min+add: the
        +2^23 magic constant rounds t_c - 0.5 to nearest -> floor(t_c),
        and min commutes with floor since 4095.5 < 4096)
  gq  = Gelu(y * -2^-10 + 8192)         (ACT; the +8192 bias cancels the
        2^23/1024 exactly in fp32, leaving Gelu(-floor-clamped/1024))
  out = (x max 0) + gq                  (DVE scalar_tensor_tensor)

The only deviation from the int32-cast reference is round-ties-to-even
when |x|*1024 is exactly an integer (~2^-13 of inputs, each off by one
4096-step table bin, ~5e-4 absolute) — negligible against the fp32 norm.
"""

import math

import numpy as np

# ---------------------------------------------------------------------------
# Problem constants (hardcoded per task contract)
# ---------------------------------------------------------------------------
N_CORES = 8
BATCH, SEQ, DMODEL = 16, 4096, 1024
SHARD_BATCH = BATCH // N_CORES  # 2
SHARD_ELEMS = SHARD_BATCH * SEQ * DMODEL  # 8388608
P = 128  # SBUF partitions
FREE = SHARD_ELEMS // P  # 65536
TILE_F = 2048  # free-dim tile width (1 MiB DMA transfers)
N_TILES = FREE // TILE_F  # 32
TABLE_SCALE_BIT = 10
TABLE_SIZE = 4096

_cached = {}


def _exact_table() -> np.ndarray:
    """T[j] = relu(k) - gelu_erf(k), k = j/1024, as float32 like the model."""
    k = np.arange(TABLE_SIZE, dtype=np.float64) / 2.0**TABLE_SCALE_BIT
    phi = np.array([0.5 * (1.0 + math.erf(v / math.sqrt(2.0))) for v in k])
    return (k - k * phi).astype(np.float32)


NBUF = 3  # SBUF double/triple buffering depth


def _build_bass(repeats: int = 1, tile_f: int = TILE_F, nbuf: int = NBUF,
                out_engine: str = "gpsimd", inplace: bool = False):
    """Build the per-core Bass module: x[128, 65536] f32 -> out[128, 65536].

    repeats > 1 re-runs the identical pass inside one NEFF (timing aid:
    device time scales with repeats while NEFF invocation overhead stays
    constant, so the difference isolates true on-silicon pass time).

    Raw Bass (no TileContext): this container's walrus encodes at most ONE
    semaphore wait per instruction, and Tile's scheduler freely emits 2-3
    (plus a many-wait tail drain), which dies in codegen with "Too many
    sync wait commands".  The pipeline here is a simple 4-stage chain, so
    manual sync with monotonic per-engine counters needs exactly one wait
    per instruction:

      SP   : dma_in(i)               waits act >= 2(i-NBUF)+2   (slot reuse)
      ACT  : t = Abs(1024 x)         waits in_sem >= 16(i+1)
      DVE  : y = min+magic-add       waits act >= 2i+1
      ACT  : gq = Gelu(-y/1024+8192) waits dve >= 2i+1
      DVE  : o = relu(x)+gq (STT)    waits act >= 2i+2  [+ standalone
                                      wait out_sem for o-slot reuse]
      POOL : dma_out(i)              waits dve >= 2i+2  (standalone wait)

    Per-engine program order supplies every other dependency.
    """
    import concourse.bass as bass
    import concourse.mybir as mybir
    from concourse.alu_op_type import AluOpType

    nc = bass.Bass(trn_type="TRN2")
    x = nc.dram_tensor("x", [P, FREE], mybir.dt.float32, kind="ExternalInput")
    out = nc.dram_tensor("out", [P, FREE], mybir.dt.float32, kind="ExternalOutput")

    f32 = mybir.dt.float32
    AF = mybir.ActivationFunctionType

    xin = nc.alloc_sbuf_tensor("xin", [P, nbuf * tile_f], f32)
    if inplace:
        # One streaming work buffer: every compute op reads and writes the
        # same tile AP (per-element read precedes write in stream order on
        # both ACT and DVE), halving SBUF so wider tiles / deeper bufs fit.
        t = y = gq = o = nc.alloc_sbuf_tensor("w", [P, nbuf * tile_f], f32)
    else:
        t = nc.alloc_sbuf_tensor("t", [P, nbuf * tile_f], f32)
        y = nc.alloc_sbuf_tensor("y", [P, nbuf * tile_f], f32)
        gq = nc.alloc_sbuf_tensor("gq", [P, nbuf * tile_f], f32)
        o = nc.alloc_sbuf_tensor("o", [P, nbuf * tile_f], f32)
    bias_t = nc.alloc_sbuf_tensor("gelu_bias", [P, 1], f32)

    s_in = nc.alloc_semaphore("s_in")
    s_act = nc.alloc_semaphore("s_act")
    s_dve = nc.alloc_semaphore("s_dve")
    s_out = nc.alloc_semaphore("s_out")
    s_boot = nc.alloc_semaphore("s_boot")

    nc.gpsimd.memset(bias_t.ap(), 8192.0).then_inc(s_boot, 1)
    nc.scalar.wait_ge(s_boot, 1)

    def buf(tensor, k):
        b = k % nbuf
        return tensor.ap()[:, b * tile_f : (b + 1) * tile_f]

    ntiles = FREE // tile_f
    for k in range(ntiles * repeats):
        i = k % ntiles
        sl = slice(i * tile_f, (i + 1) * tile_f)

        # SP: load tile.  Slot reuse: xin[b] last read by DVE.stt(k-NBUF)
        # -> wait dve >= 2(k-NBUF)+2.
        dma_in = nc.sync.dma_start(out=buf(xin, k), in_=x[:, sl])
        dma_in.then_inc(s_in, 16)
        if k >= nbuf:
            dma_in._wait_ge(s_dve, 2 * (k - nbuf) + 2)

        # ACT: t = |x| * 1024   (exact power-of-two scale)
        if inplace and k >= nbuf:
            # w[b] slot reuse vs dma_out(k-nbuf) (first writer is Abs here)
            nc.scalar.wait_ge(s_out, 16 * (k - nbuf + 1))
        act_abs = nc.scalar.activation(buf(t, k), buf(xin, k), AF.Abs, scale=1024.0)
        act_abs._wait_ge(s_in, 16 * (k + 1))
        act_abs.then_inc(s_act, 1)  # -> 2k+1

        # DVE: y = min(t, 4095.5) + (2^23 - 0.5)  == floor(min(t,4095.5)) + 2^23
        # (RNE magic rounding; min commutes with floor below 4096)
        dve_ts = nc.vector.tensor_scalar(
            out=buf(y, k), in0=buf(t, k),
            scalar1=4095.5, scalar2=float(2.0**23) - 0.5,
            op0=AluOpType.min, op1=AluOpType.add,
        )
        dve_ts._wait_ge(s_act, 2 * k + 1)
        dve_ts.then_inc(s_dve, 1)  # -> 2k+1

        # ACT: gq = Gelu(y * -2^-10 + 8192) = Gelu(-c/1024) = -table[c]
        act_gelu = nc.scalar.activation(
            buf(gq, k), buf(y, k), AF.Gelu,
            bias=bias_t.ap()[:, :], scale=-(2.0**-TABLE_SCALE_BIT),
        )
        act_gelu._wait_ge(s_dve, 2 * k + 1)
        act_gelu.then_inc(s_act, 1)  # -> 2k+2

        # DVE: o = (x max 0) + gq = relu(x) - table[c]
        if not inplace and k >= nbuf:
            # o[b] slot reuse vs dma_out(k-nbuf)
            nc.vector.wait_ge(s_out, 16 * (k - nbuf + 1))
        dve_stt = nc.vector.scalar_tensor_tensor(
            out=buf(o, k), in0=buf(xin, k), scalar=0.0, in1=buf(gq, k),
            op0=AluOpType.max, op1=AluOpType.add,
        )
        dve_stt._wait_ge(s_act, 2 * k + 2)
        dve_stt.then_inc(s_dve, 1)  # -> 2k+2

        # store tile (SWDGE on gpsimd by default; ACT-HWDGE as variant)
        out_eng = nc.gpsimd if out_engine == "gpsimd" else nc.scalar
        out_eng.wait_ge(s_dve, 2 * k + 2)
        out_eng.dma_start(out=out[:, sl], in_=buf(o, k)).then_inc(s_out, 16)

    nc.sync.wait_ge(s_out, 16 * ntiles * repeats)
    return nc


def _get_nc(repeats: int = 1):
    key = ("nc", repeats)
    if key not in _cached:
        _cached[key] = _build_bass(repeats)
    return _cached[key]


def _run_device(x_np: np.ndarray, trace: bool = False):
    """Shard x over 8 cores, run the Bass kernel, gather. Returns (out, results)."""
    from concourse.bass_utils import run_bass_kernel_spmd

    nc = _get_nc()
    shards = [
        np.ascontiguousarray(
            x_np[i * SHARD_BATCH : (i + 1) * SHARD_BATCH].reshape(P, FREE)
        )
        for i in range(N_CORES)
    ]
    in_maps = [{"x": s} for s in shards]
    res = run_bass_kernel_spmd(
        nc, in_maps, core_ids=list(range(N_CORES)), trace=trace
    )
    out = np.empty((BATCH, SEQ, DMODEL), dtype=np.float32)
    for i, r in enumerate(res.results):
        out[i * SHARD_BATCH : (i + 1) * SHARD_BATCH] = r["out"].reshape(
            SHARD_BATCH, SEQ, DMODEL
        )
    return out, res


def _host_reference(x: np.ndarray, table: np.ndarray) -> np.ndarray:
    a = np.abs(x)
    c = np.minimum((a * 2.0**TABLE_SCALE_BIT).astype(np.int32), TABLE_SIZE - 1)
    return np.where(x >= 0, x, 0.0).astype(np.float32) - table[c]


def kernel(x: np.ndarray, table: np.ndarray) -> np.ndarray:
    x = np.asarray(x, dtype=np.float32)
    table = np.asarray(table, dtype=np.float32)
    assert x.shape == (BATCH, SEQ, DMODEL), x.shape
    assert table.shape == (TABLE_SIZE,), table.shape

    # The device path encodes -table[c] as Gelu(-c/1024): valid iff the
    # runtime table is the erf-GELU difference table the model uses.
    if "exact_table" not in _cached:
        _cached["exact_table"] = _exact_table()
    if not np.max(np.abs(table - _cached["exact_table"])) < 1e-5:
        # Arbitrary table: no line-rate device gather exists; stay exact.
        return _host_reference(x, table)

    out, _ = _run_device(x, trace=False)
    return out
